# revision 22
# baseline (speedup 1.0000x reference)
"""GIN encoder (3-layer GINConv + BN + residual VQ) on 8 Trainium2 cores.

Sharding: nodes/edges partitioned by dst node id across 8 cores; the full
node-feature table is replicated per layer via AllGather so each core can
gather arbitrary src rows.  segment_sum is done as: indirect-DMA gather of
src rows (128-edge chunks) + one-hot scatter matmul on the TensorEngine
accumulating into PSUM per 128-node dst window.  BatchNorm statistics use a
tiny AllReduce; normalization is folded into the next layer's first Linear
(row-scaled W1 + rank-1 degree term) so the AllGather ships raw activations
and never waits on the stats.  The residual-VQ runs entirely on-device and
overlaps the AllGather.  Host code does edge preprocessing, codebook
normalization, and output reassembly (xs/xpool BN affine, id repacking).
"""

import hashlib
import math
import os
import sys

import numpy as np

for _p in ("/opt/trn_rl_repo", os.path.expanduser("~/.axon_site/_ro/trn_rl_repo")):
    if os.path.isdir(_p) and _p not in sys.path:
        sys.path.insert(0, _p)
        break

import concourse.bass as bass
import concourse.bacc as bacc
import concourse.mybir as mybir
import concourse.tile as tile
from concourse.bass_utils import run_bass_kernel_spmd
from concourse.masks import make_identity

P = 128
NCORES = 8
L = 3          # GNN layers
R = 3          # residual VQ depth
K = 16         # codebook size
BN_EPS = 1e-5
COMMIT_W = 0.25
BIG = 1.0e8

F32 = mybir.dt.float32
BF16 = mybir.dt.bfloat16
I32 = mybir.dt.int32

# gather/allgather dtype for the replicated node table ("f32" or "bf16")
GATHER_DTYPE = os.environ.get("KERNEL_GATHER_DTYPE", "f32")

AL = mybir.AluOpType
AX = mybir.AxisListType


# --------------------------------------------------------------------------
# host-side preprocessing
# --------------------------------------------------------------------------

def _prep(x, edge_index, batch):
    """Partition/sort/pad edges per core; build device-side index arrays.

    Edges are grouped by (dst window, src quadrant); each group is padded to
    whole 128-edge chunks with counts shared across cores (SPMD).  Chunks are
    laid out globally as [window-group][quadrant][window][chunk] so that one
    dma_gather per (window-group, quadrant) fetches everything, with int16
    indices local to the quadrant.
    """
    N, D = x.shape
    assert N % NCORES == 0, N
    NB = N // NCORES
    NTILE = (NB + P - 1) // P
    NBP = NTILE * P
    QS = (N + 3) // 4
    assert QS <= 32767, QS

    src = np.ascontiguousarray(edge_index[0]).astype(np.int64)
    dst = np.ascontiguousarray(edge_index[1]).astype(np.int64)
    # self edges implement the "+ h" (GIN eps=0) term
    allsrc = np.concatenate([src, np.arange(N, dtype=np.int64)])
    alldst = np.concatenate([dst, np.arange(N, dtype=np.int64)])

    core = alldst // NB
    win = (alldst % NB) // P
    quad = allsrc // QS
    order = np.lexsort((allsrc, quad, win, core))
    s_src = allsrc[order]
    s_core = core[order]
    s_win = win[order]
    s_quad = quad[order]
    s_dl = (alldst[order] % NB) % P

    # chunk counts per (window, quadrant), shared across cores
    cnt3d = np.zeros((NCORES, NTILE, 4), np.int64)
    np.add.at(cnt3d, (s_core, s_win, s_quad), 1)
    CWQ = ((cnt3d.max(axis=0) + P - 1) // P).astype(np.int64)  # [NTILE, 4]
    CW = CWQ.sum(axis=1)                                        # chunks per window

    # window groups sized to ~GBUDGET chunks
    GBUDGET = 84
    groups = []
    cur = []
    acc = 0
    for w in range(NTILE):
        if cur and acc + CW[w] > GBUDGET:
            groups.append(cur)
            cur = []
            acc = 0
        cur.append(w)
        acc += int(CW[w])
    if cur:
        groups.append(cur)

    # global chunk layout + gather call table
    chunk_cols = {}   # (w) -> list of global chunk columns in matmul order
    calls = []        # (quad, idx_col_off, n_idx, gchunk_off, n_chunks, grp_id)
    grp_meta = []     # (gchunk_off, n_chunks) per group
    gch = 0
    icol = 0
    for gi, grp in enumerate(groups):
        g0 = gch
        for w in grp:
            chunk_cols[w] = []
        for q in range(4):
            nch = int(CWQ[list(grp), q].sum() if False else sum(int(CWQ[w][q]) for w in grp))
            if nch == 0:
                continue
            calls.append(dict(grp=gi, quad=q, icol=icol, nidx=nch * P,
                              goff=gch, nch=nch))
            for w in grp:
                for _ in range(int(CWQ[w][q])):
                    chunk_cols[w].append(gch)
                    gch += 1
            icol += nch * P // 16
        grp_meta.append((g0, gch - g0))
    TOTCHUNK = gch
    TOTI16 = icol

    # slot of each edge inside its (core, window, quadrant) group
    flat_gid = (s_core * NTILE + s_win) * 4 + s_quad
    gcnt = cnt3d.reshape(-1)
    gstart = np.zeros(NCORES * NTILE * 4, np.int64)
    gstart[1:] = np.cumsum(gcnt)[:-1]
    kpos = np.arange(len(s_src)) - gstart[flat_gid]

    # map (w, q, chunk-in-group) -> global chunk id
    wq_base = np.zeros((NTILE, 4), np.int64)
    for w in range(NTILE):
        pos = 0
        cols = chunk_cols[w]
        for q in range(4):
            if CWQ[w][q]:
                wq_base[w, q] = cols[pos]
                pos += int(CWQ[w][q])
    # global chunk ids are contiguous per (w, q) run
    e_gchunk = wq_base[s_win, s_quad] + kpos // P
    e_slot = kpos % P

    dst_arr = np.full((NCORES, P, TOTCHUNK), -1.0, np.float32)
    dst_arr[s_core, e_slot, e_gchunk] = s_dl.astype(np.float32)

    # int16 index array in dma_gather layout (16-partition wrap, replicated x8)
    idx16 = np.zeros((NCORES, 16, TOTI16), np.int16)
    # flat position within the call = (e_gchunk - call.goff)*128 + e_slot
    call_goff = np.zeros(TOTCHUNK, np.int64)
    call_icol = np.zeros(TOTCHUNK, np.int64)
    for cl in calls:
        call_goff[cl["goff"]:cl["goff"] + cl["nch"]] = cl["goff"]
        call_icol[cl["goff"]:cl["goff"] + cl["nch"]] = cl["icol"]
    f = (e_gchunk - call_goff[e_gchunk]) * P + e_slot
    idx16[s_core, f % 16, call_icol[e_gchunk] + f // 16] = (s_src % QS).astype(np.int16)
    idx16 = np.tile(idx16, (1, 8, 1))  # replicate across the 8 gpsimd cores

    # in-degree + 1 (self) per node, padded to NBP, laid out as a row
    deg = np.bincount(alldst, minlength=N).astype(np.float32)
    cnt_row = np.zeros((NCORES, 1, NBP), np.float32)
    cnt_row[:, 0, :NB] = deg.reshape(NCORES, NB)

    # graph ids: local per core
    b = np.ascontiguousarray(batch).astype(np.int64)
    assert np.all(np.diff(b) >= 0), "batch must be sorted"
    g_lo = b[np.arange(NCORES) * NB]
    bcol = np.full((NCORES, P, NTILE), 127.0, np.float32)
    loc = (b.reshape(NCORES, NB) - g_lo[:, None]).astype(np.float32)
    g_span = loc.max(axis=1).astype(np.int64) + 1
    assert g_span.max() <= P, g_span
    for c in range(NCORES):
        padded = np.full(NBP, 127.0, np.float32)
        padded[:NB] = loc[c]
        bcol[c] = padded.reshape(NTILE, P).T

    return dict(
        N=N, D=D, NB=NB, NTILE=NTILE, NBP=NBP, QS=QS,
        TOTCHUNK=TOTCHUNK, TOTI16=TOTI16,
        groups=groups, calls=calls, grp_meta=grp_meta,
        chunk_cols={w: list(map(int, v)) for w, v in chunk_cols.items()},
        idx16=idx16, dst_arr=dst_arr, cnt_row=cnt_row, bcol=bcol,
        g_lo=g_lo, g_span=g_span, batch=b,
    )


def _l2norm_np(v):
    return v * (1.0 / np.sqrt(np.sum(v * v, axis=-1, keepdims=True) + 1e-12))


# --------------------------------------------------------------------------
# device program
# --------------------------------------------------------------------------

def _build(meta):
    N, D, NB = meta["N"], meta["D"], meta["NB"]
    NTILE, NBP, QS = meta["NTILE"], meta["NBP"], meta["QS"]
    TOTCHUNK, TOTI16 = meta["TOTCHUNK"], meta["TOTI16"]
    groups, calls, grp_meta = meta["groups"], meta["calls"], meta["grp_meta"]
    chunk_cols = meta["chunk_cols"]
    GCHMAX = max(n for _, n in grp_meta)
    SWMAX = max(len(v) for v in chunk_cols.values())
    ICMAX = max(cl["nidx"] // 16 for cl in calls)
    calls_by_grp = {}
    for cl in calls:
        calls_by_grp.setdefault(cl["grp"], []).append(cl)
    GD = F32 if GATHER_DTYPE == "f32" else BF16
    I16 = mybir.dt.int16
    rg = [list(range(NCORES))]

    nc = bacc.Bacc(None)

    htab = nc.dram_tensor("htab", [N, D], GD, kind="ExternalInput")
    idx16t = nc.dram_tensor("idx16t", [P, TOTI16], I16, kind="ExternalInput")
    dstl = nc.dram_tensor("dstl", [P, TOTCHUNK], F32, kind="ExternalInput")
    cntr = nc.dram_tensor("cntr", [1, NBP], F32, kind="ExternalInput")
    bcolt = nc.dram_tensor("bcolt", [P, NTILE], F32, kind="ExternalInput")
    w1t = nc.dram_tensor("w1t", [L, D, D], F32, kind="ExternalInput")
    w2t = nc.dram_tensor("w2t", [L, D, D], F32, kind="ExternalInput")
    cvect = nc.dram_tensor("cvect", [P, 12], F32, kind="ExternalInput")
    cbnt = nc.dram_tensor("cbnt", [L * R, K, D], F32, kind="ExternalInput")
    cbnTt = nc.dram_tensor("cbnTt", [L * R, D, K], F32, kind="ExternalInput")

    xs_raw = nc.dram_tensor("xs_raw", [NB, L * D], F32, kind="ExternalOutput")
    ids_pack = nc.dram_tensor("ids_pack", [P, NTILE * L * R], I32, kind="ExternalOutput")
    xpool_raw = nc.dram_tensor("xpool_raw", [L * P, P], F32, kind="ExternalOutput")
    at_out = nc.dram_tensor("at_out", [P, 2 * L], F32, kind="ExternalOutput")
    commit_out = nc.dram_tensor("commit_out", [P, 1], F32, kind="ExternalOutput")

    stage = nc.dram_tensor("stage", [NB, D], GD)
    zdram = nc.dram_tensor("zdram", [P, NBP], F32)
    hf = [None,
          nc.dram_tensor("hf1", [N, D], GD, addr_space="Shared"),
          nc.dram_tensor("hf2", [N, D], GD, addr_space="Shared")]
    st_in = nc.dram_tensor("st_in", [P, 2], F32)
    st_out = nc.dram_tensor("st_out", [P, 2], F32, addr_space="Shared")

    from contextlib import ExitStack
    with tile.TileContext(nc) as tc, ExitStack() as ctx:
        cpool = ctx.enter_context(tc.tile_pool(name="consts", bufs=1))
        gpool = ctx.enter_context(tc.tile_pool(name="gath", bufs=2))
        spool = ctx.enter_context(tc.tile_pool(name="sel", bufs=2))
        mpool = ctx.enter_context(tc.tile_pool(name="work", bufs=4))
        vpool = ctx.enter_context(tc.tile_pool(name="vq", bufs=3))
        pp = ctx.enter_context(tc.tile_pool(name="ps", bufs=6, space="PSUM"))
        xpp = ctx.enter_context(tc.tile_pool(name="xp", bufs=1, space="PSUM"))

        # ---- constants ----
        ident = cpool.tile([P, P], F32)
        make_identity(nc, ident[:])
        iota_i = cpool.tile([P, P], I32)
        nc.gpsimd.iota(iota_i[:], pattern=[[1, P]], base=0, channel_multiplier=0)
        iota_row = cpool.tile([P, P], F32)
        nc.vector.tensor_copy(iota_row[:], iota_i[:])
        iota16_i = cpool.tile([P, K], I32)
        nc.gpsimd.iota(iota16_i[:], pattern=[[1, K]], base=0, channel_multiplier=0)
        iota16 = cpool.tile([P, K], F32)
        nc.vector.tensor_copy(iota16[:], iota16_i[:])

        w1_sb = []
        w2_sb = []
        for i in range(L):
            w1i = cpool.tile([P, P], F32, name=f"w1_{i}")
            nc.sync.dma_start(out=w1i[:], in_=w1t[i])
            w1_sb.append(w1i)
            w2i = cpool.tile([P, P], F32, name=f"w2_{i}")
            nc.sync.dma_start(out=w2i[:], in_=w2t[i])
            w2_sb.append(w2i)
        cvec = cpool.tile([P, 12], F32)
        nc.sync.dma_start(out=cvec[:], in_=cvect[:])
        cbn_sb = []
        cbnT_sb = []
        for k in range(L * R):
            cb = cpool.tile([K, D], F32, name=f"cbn_{k}")
            nc.sync.dma_start(out=cb[:], in_=cbnt[k])
            cbn_sb.append(cb)
            cbT = cpool.tile([D, K], F32, name=f"cbnT_{k}")
            nc.sync.dma_start(out=cbT[:], in_=cbnTt[k])
            cbnT_sb.append(cbT)
        dst_sb = cpool.tile([P, TOTCHUNK], F32)
        nc.sync.dma_start(out=dst_sb[:], in_=dstl[:])
        bcol_sb = cpool.tile([P, NTILE], F32)
        nc.sync.dma_start(out=bcol_sb[:], in_=bcolt[:])

        eps_col = cpool.tile([P, 1], F32)
        nc.vector.memset(eps_col[:], BN_EPS)

        ssum = cpool.tile([P, NTILE], F32)
        ssq = cpool.tile([P, NTILE], F32)
        commit_cols = cpool.tile([P, L * R * NTILE], F32)
        nc.vector.memset(commit_cols[:], 0.0)
        ids_sb = cpool.tile([P, NTILE * L * R], I32)
        nc.gpsimd.memset(ids_sb[:], 0)
        at_sb = cpool.tile([P, 2 * L], F32)

        atpool = ctx.enter_context(tc.tile_pool(name="atp", bufs=2))
        a_prev = t_prev = None
        u_row = None

        for i in range(L):
            table = htab if i == 0 else hf[i]

            # effective first-layer weight: W1eff = diag(a_prev) @ W1
            if i == 0:
                w1e = w1_sb[0]
            else:
                w1e = atpool.tile([P, P], F32, name="w1e", tag="w1e")
                nc.vector.tensor_scalar(
                    out=w1e[:], in0=w1_sb[i][:], scalar1=a_prev[:, :1],
                    scalar2=None, op0=AL.mult)
                up = pp.tile([1, P], F32, tag="ps", padded_shape=[P, P])
                nc.tensor.matmul(out=up[:1, :], lhsT=t_prev[:, :1],
                                 rhs=w1_sb[i][:], start=True, stop=True)
                u_row = atpool.tile([1, P], F32, name="u_row", tag="u_row")
                nc.scalar.copy(out=u_row[:1, :], in_=up[:1, :])

            xpool_ps = xpp.tile([P, P], F32, name="xpool_ps", tag="xp")

            # ---------------- phase A: gather + scatter-matmul + MLP ----------
            for gi, grp in enumerate(groups):
                g0, gnch = grp_meta[gi]
                Gg = gpool.tile([P, GCHMAX * P], GD, name="Gg", tag="G")
                for cl in calls_by_grp.get(gi, []):
                    nic = cl["nidx"] // 16
                    it = spool.tile([P, ICMAX], I16, name="it", tag="it")
                    nc.sync.dma_start(out=it[:, :nic],
                                      in_=idx16t[:, cl["icol"]:cl["icol"] + nic])
                    qb = cl["quad"] * QS
                    qn = min(QS, N - qb)
                    nc.gpsimd.dma_gather(
                        out_ap=Gg[:, (cl["goff"] - g0) * P:
                                  (cl["goff"] - g0 + cl["nch"]) * P]
                        .rearrange("p (c k) -> p c k", k=D),
                        in_ap=table[qb:qb + qn, :],
                        idxs_ap=it[:, :nic],
                        num_idxs=cl["nidx"], num_idxs_reg=cl["nidx"],
                        elem_size=D, single_packet=False)
                for w in grp:
                    cols = chunk_cols[w]
                    cw = len(cols)
                    wn = min(P, NB - w * P)
                    S = spool.tile([P, SWMAX * P], GD, name="S", tag="S")
                    for j, gc in enumerate(cols):
                        nc.vector.tensor_scalar(
                            out=S[:, j * P:(j + 1) * P], in0=iota_row[:],
                            scalar1=dst_sb[:, gc:gc + 1], scalar2=None,
                            op0=AL.is_equal)
                    agg = pp.tile([P, P], F32, name="agg", tag="ps")
                    for j, gc in enumerate(cols):
                        nc.tensor.matmul(
                            out=agg[:], lhsT=Gg[:, (gc - g0) * P:(gc - g0 + 1) * P],
                            rhs=S[:, j * P:(j + 1) * P],
                            start=(j == 0), stop=(j == cw - 1))
                    A0 = mpool.tile([P, P], F32, name="A0", tag="m")
                    nc.scalar.copy(out=A0[:], in_=agg[:])
                    h1p = pp.tile([P, P], F32, name="h1p", tag="ps")
                    nc.tensor.matmul(out=h1p[:], lhsT=w1e[:], rhs=A0[:],
                                     start=True, stop=(i == 0))
                    if i > 0:
                        ct = vpool.tile([1, P], F32, name="ct", tag="ct")
                        nc.sync.dma_start(out=ct[:1, :],
                                          in_=cntr[:1, w * P:(w + 1) * P])
                        nc.tensor.matmul(out=h1p[:], lhsT=u_row[:1, :],
                                         rhs=ct[:1, :],
                                         start=False, stop=True)
                    H1 = mpool.tile([P, P], F32, name="H1", tag="m")
                    nc.vector.tensor_scalar(out=H1[:], in0=h1p[:],
                                            scalar1=cvec[:, i:i + 1], scalar2=0.0,
                                            op0=AL.add, op1=AL.max)
                    z2p = pp.tile([P, P], F32, name="z2p", tag="ps")
                    nc.tensor.matmul(out=z2p[:], lhsT=w2_sb[i][:], rhs=H1[:],
                                     start=True, stop=True)
                    zt0 = mpool.tile([P, P], F32, name="zt0", tag="m")
                    nc.vector.tensor_scalar(out=zt0[:], in0=z2p[:],
                                            scalar1=cvec[:, 3 + i:4 + i],
                                            scalar2=0.0,
                                            op0=AL.add, op1=AL.max)
                    zsl = zt0[:, :wn]
                    nc.sync.dma_start(out=zdram[:, w * P:(w + 1) * P], in_=zt0[:])
                    # channel stats (partitions = channels, free = nodes)
                    nc.vector.tensor_reduce(out=ssum[:, w:w + 1], in_=zsl,
                                            op=AL.add, axis=AX.X)
                    sq = mpool.tile([P, P], F32, name="sq", tag="m")
                    nc.vector.scalar_tensor_tensor(
                        out=sq[:, :wn], in0=zsl, scalar=1.0, in1=zsl,
                        op0=AL.mult, op1=AL.mult, accum_out=ssq[:, w:w + 1])
                    # node-major transpose for xs / stage / xpool
                    ztp = pp.tile([P, P], F32, name="ztp", tag="ps")
                    nc.tensor.transpose(out=ztp[:wn, :], in_=zsl, identity=ident[:])
                    zt = mpool.tile([P, P], F32, name="zt", tag="m")
                    nc.scalar.copy(out=zt[:wn, :], in_=ztp[:wn, :])
                    nc.sync.dma_start(
                        out=xs_raw[w * P:w * P + wn, i * D:(i + 1) * D],
                        in_=zt[:wn, :])
                    if i < L - 1:
                        if GD == F32:
                            nc.sync.dma_start(out=stage[w * P:w * P + wn, :],
                                              in_=zt[:wn, :])
                        else:
                            nc.gpsimd.dma_start(out=stage[w * P:w * P + wn, :],
                                                in_=zt[:wn, :])
                    B = mpool.tile([P, P], F32, name="B", tag="m")
                    nc.vector.tensor_scalar(out=B[:], in0=iota_row[:],
                                            scalar1=bcol_sb[:, w:w + 1],
                                            scalar2=None, op0=AL.is_equal)
                    nc.tensor.matmul(out=xpool_ps[:], lhsT=B[:wn, :],
                                     rhs=zt[:wn, :],
                                     start=(w == 0), stop=(w == NTILE - 1),
                                     skip_group_check=True)

            # ---------------- stats: reduce + AllReduce + a/t -----------------
            s1 = vpool.tile([P, 2], F32, name="s1", tag="s1")
            nc.vector.tensor_reduce(out=s1[:, 0:1], in_=ssum[:, :NTILE],
                                    op=AL.add, axis=AX.X)
            nc.vector.tensor_reduce(out=s1[:, 1:2], in_=ssq[:, :NTILE],
                                    op=AL.add, axis=AX.X)
            nc.sync.dma_start(out=st_in[:], in_=s1[:])
            nc.gpsimd.collective_compute(
                "AllReduce", AL.add, replica_groups=rg,
                ins=[st_in[:]], outs=[st_out[:]])
            if i < L - 1:
                nc.gpsimd.collective_compute(
                    "AllGather", AL.bypass, replica_groups=rg,
                    ins=[stage[:]], outs=[hf[i + 1][:]])
            s2 = vpool.tile([P, 2], F32, name="s2", tag="s2")
            nc.sync.dma_start(out=s2[:], in_=st_out[:])
            mean = vpool.tile([P, 1], F32, name="mean", tag="mean")
            nc.vector.tensor_scalar(out=mean[:], in0=s2[:, 0:1], scalar1=1.0 / N,
                                    scalar2=None, op0=AL.mult)
            var = vpool.tile([P, 1], F32, name="var", tag="var")
            # var = E[x^2] - mean^2  (biased)
            nc.vector.scalar_tensor_tensor(
                out=var[:], in0=mean[:], scalar=-1.0, in1=mean[:],
                op0=AL.mult, op1=AL.mult)
            nc.vector.scalar_tensor_tensor(
                out=var[:], in0=s2[:, 1:2], scalar=1.0 / N, in1=var[:],
                op0=AL.mult, op1=AL.add)
            sd = vpool.tile([P, 1], F32, name="sd", tag="sd")
            nc.scalar.activation(out=sd[:], in_=var[:],
                                 func=mybir.ActivationFunctionType.Sqrt,
                                 bias=eps_col[:, :1])
            inv = vpool.tile([P, 1], F32, name="inv", tag="inv")
            nc.vector.reciprocal(out=inv[:], in_=sd[:])
            a_col = atpool.tile([P, 1], F32, name="a_col", tag="a_col")
            nc.vector.tensor_tensor(out=a_col[:], in0=inv[:],
                                    in1=cvec[:, 6 + i:7 + i], op=AL.mult)
            t_col = atpool.tile([P, 1], F32, name="t_col", tag="t_col")
            # t = beta - mean*a
            nc.vector.scalar_tensor_tensor(
                out=t_col[:], in0=mean[:], scalar=-1.0, in1=a_col[:],
                op0=AL.mult, op1=AL.mult)
            nc.vector.tensor_tensor(out=t_col[:], in0=t_col[:],
                                    in1=cvec[:, 9 + i:10 + i], op=AL.add)
            nc.vector.tensor_copy(out=at_sb[:, 2 * i:2 * i + 1], in_=a_col[:])
            nc.vector.tensor_copy(out=at_sb[:, 2 * i + 1:2 * i + 2], in_=t_col[:])
            a_prev, t_prev = a_col, t_col

            # xpool out
            xpc = mpool.tile([P, P], F32, name="xpc", tag="m")
            nc.scalar.copy(out=xpc[:], in_=xpool_ps[:])
            nc.sync.dma_start(out=xpool_raw[i * P:(i + 1) * P, :], in_=xpc[:])

            # ---------------- phase D: residual VQ ---------------------------
            for w in range(NTILE):
                wn = min(P, NB - w * P)
                zr = mpool.tile([P, P], F32, name="zr", tag="m")
                nc.sync.dma_start(out=zr[:], in_=zdram[:, w * P:(w + 1) * P])
                r = mpool.tile([P, P], F32, name="r", tag="r")
                nc.vector.tensor_scalar(out=r[:, :wn], in0=zr[:, :wn],
                                        scalar1=a_col[:, :1], scalar2=t_col[:, :1],
                                        op0=AL.mult, op1=AL.add)
                for l in range(R):
                    kk = i * R + l
                    simp = pp.tile([P, P], F32, name="simp", tag="ps")
                    nc.tensor.matmul(out=simp[:wn, :K], lhsT=r[:, :wn],
                                     rhs=cbnT_sb[kk][:, :], start=True, stop=True)
                    mcol = vpool.tile([P, 1], F32, name="mcol", tag="mcol")
                    nc.vector.tensor_reduce(out=mcol[:wn, :], in_=simp[:wn, :K],
                                            op=AL.max, axis=AX.X)
                    v = vpool.tile([P, K], F32, name="v", tag="v")
                    nc.vector.tensor_scalar(out=v[:wn, :], in0=simp[:wn, :K],
                                            scalar1=mcol[:wn, :1], scalar2=-BIG,
                                            op0=AL.subtract, op1=AL.mult)
                    nc.vector.tensor_tensor(out=v[:wn, :], in0=v[:wn, :],
                                            in1=iota16[:wn, :], op=AL.add)
                    idxc = vpool.tile([P, 1], F32, name="idxc", tag="idxc")
                    nc.vector.tensor_reduce(out=idxc[:wn, :], in_=v[:wn, :],
                                            op=AL.min, axis=AX.X)
                    oh = vpool.tile([P, K], F32, name="oh", tag="oh")
                    nc.vector.tensor_scalar(out=oh[:wn, :], in0=iota16[:wn, :],
                                            scalar1=idxc[:wn, :1], scalar2=None,
                                            op0=AL.is_equal)
                    ohtp = pp.tile([K, P], F32, name="ohtp", tag="ps",
                                   padded_shape=[P, P])
                    nc.tensor.transpose(out=ohtp[:K, :wn], in_=oh[:wn, :K],
                                        identity=ident[:wn, :wn])
                    oht = vpool.tile([K, P], F32, name="oht", tag="oht")
                    nc.scalar.copy(out=oht[:K, :wn], in_=ohtp[:K, :wn])
                    qp = pp.tile([P, P], F32, name="qp", tag="ps")
                    nc.tensor.matmul(out=qp[:, :wn], lhsT=cbn_sb[kk][:, :],
                                     rhs=oht[:K, :wn], start=True, stop=True)
                    nc.vector.tensor_tensor(out=r[:, :wn], in0=r[:, :wn],
                                            in1=qp[:, :wn], op=AL.subtract)
                    sq2 = mpool.tile([P, P], F32, name="sq2", tag="m")
                    ccol = kk * NTILE + w
                    nc.vector.scalar_tensor_tensor(
                        out=sq2[:, :wn], in0=r[:, :wn], scalar=1.0, in1=r[:, :wn],
                        op0=AL.mult, op1=AL.mult,
                        accum_out=commit_cols[:, ccol:ccol + 1])
                    icol = w * (L * R) + kk
                    nc.vector.tensor_copy(out=ids_sb[:wn, icol:icol + 1],
                                          in_=idxc[:wn, :1])

        # ---- finals ----
        cto = vpool.tile([P, 1], F32, name="cto", tag="cto")
        nc.vector.tensor_reduce(out=cto[:], in_=commit_cols[:, :],
                                op=AL.add, axis=AX.X)
        nc.sync.dma_start(out=commit_out[:], in_=cto[:])
        nc.sync.dma_start(out=ids_pack[:], in_=ids_sb[:])
        nc.sync.dma_start(out=at_out[:], in_=at_sb[:])

    nc.finalize()
    return nc


# --------------------------------------------------------------------------
# driver
# --------------------------------------------------------------------------

_CACHE = {}


def _ensure_ntff_hook():
    """bass_utils' trace=True path imports antenv.axon_hooks, which this
    image lacks; synthesize it and install the ctypes NTFF hook."""
    import types
    try:
        import antenv.axon_hooks  # noqa: F401
        return
    except ImportError:
        pass
    mod = types.ModuleType("antenv.axon_hooks")
    _h = {"v": None}
    mod.set_axon_ntff_profile_hook = lambda h: _h.__setitem__("v", h)
    mod.get_axon_ntff_profile_hook = lambda: _h["v"]
    sys.modules["antenv.axon_hooks"] = mod
    try:
        import antenv
        antenv.axon_hooks = mod
    except ImportError:
        pass
    try:
        site_dir = os.path.expanduser("~/.axon_site")
        if site_dir not in sys.path:
            sys.path.insert(0, site_dir)
        from trn_agent_boot.trn_boot import _ntff_profile_via_ctypes
        mod.set_axon_ntff_profile_hook(
            _ntff_profile_via_ctypes("/opt/axon/libaxon_pjrt.so"))
    except Exception:
        pass


def _get_program(meta, fingerprint):
    if _CACHE.get("fp") != fingerprint:
        _CACHE["fp"] = fingerprint
        _CACHE["nc"] = _build(meta)
    return _CACHE["nc"]


def kernel(x, edge_index, batch, W1, b1, W2, b2, gamma, beta, codebooks,
           _sim=False):
    x = np.asarray(x, np.float32)
    edge_index = np.asarray(edge_index)
    batch = np.asarray(batch)
    W1 = np.asarray(W1, np.float32)
    b1 = np.asarray(b1, np.float32)
    W2 = np.asarray(W2, np.float32)
    b2 = np.asarray(b2, np.float32)
    gamma = np.asarray(gamma, np.float32)
    beta = np.asarray(beta, np.float32)
    codebooks = np.asarray(codebooks, np.float32)

    N, D = x.shape
    meta = _prep(x, edge_index, batch)
    NB, NTILE, NBP = meta["NB"], meta["NTILE"], meta["NBP"]
    n_graphs = int(batch.max()) + 1

    GD_np = np.float32 if GATHER_DTYPE == "f32" else np.dtype("bfloat16")
    try:
        htab_np = x.astype(GD_np)
    except TypeError:
        import ml_dtypes
        htab_np = x.astype(ml_dtypes.bfloat16)

    # packed per-channel constants: b1 x3 | b2 x3 | gamma x3 | beta x3
    cvec_np = np.stack([b1[0], b1[1], b1[2], b2[0], b2[1], b2[2],
                        gamma[0], gamma[1], gamma[2],
                        beta[0], beta[1], beta[2]], axis=1).astype(np.float32)
    cbn_np = _l2norm_np(codebooks.reshape(L * R, K, D)).astype(np.float32)
    cbnT_np = np.ascontiguousarray(cbn_np.transpose(0, 2, 1))

    fingerprint = hashlib.sha1(
        b"v2" + str((N, D, meta["TOTCHUNK"])).encode()
        + edge_index.tobytes() + batch.tobytes()
    ).hexdigest()
    nc = _get_program(meta, fingerprint)

    in_maps = []
    for c in range(NCORES):
        in_maps.append({
            "htab": htab_np,
            "idx16t": meta["idx16"][c],
            "dstl": meta["dst_arr"][c],
            "cntr": meta["cnt_row"][c],
            "bcolt": meta["bcol"][c],
            "w1t": W1, "w2t": W2, "cvect": cvec_np,
            "cbnt": cbn_np, "cbnTt": cbnT_np,
        })

    if _sim:
        from concourse.bass_interp import MultiCoreSim
        sim = MultiCoreSim(nc, NCORES)
        for c in range(NCORES):
            for k2, v in in_maps[c].items():
                sim.cores[c].tensor(k2)[:] = np.ascontiguousarray(v)
        sim.simulate()
        results = []
        for c in range(NCORES):
            results.append({nm: np.array(sim.cores[c].tensor(nm))
                            for nm in ["xs_raw", "ids_pack", "xpool_raw",
                                       "at_out", "commit_out"]})
    else:
        trace = os.environ.get("KERNEL_TRACE") == "1"
        if trace:
            _ensure_ntff_hook()
        br = run_bass_kernel_spmd(nc, in_maps, list(range(NCORES)), trace=trace)
        results = br.results
        if trace:
            _CACHE["exec_ns"] = br.exec_time_ns

    return _assemble(results, meta, n_graphs)


def _assemble(results, meta, n_graphs):
    N, D, NB, NTILE = meta["N"], meta["D"], meta["NB"], meta["NTILE"]
    batch = meta["batch"]
    at = results[0]["at_out"].astype(np.float32)       # [128, 6]
    a = np.stack([at[:, 2 * i] for i in range(L)])      # [L, D]
    t = np.stack([at[:, 2 * i + 1] for i in range(L)])  # [L, D]

    xs_cat = np.concatenate([r["xs_raw"] for r in results], axis=0)
    for i in range(L):
        blk = xs_cat[:, i * D:(i + 1) * D]
        np.multiply(blk, a[i][None, :], out=blk)
        np.add(blk, t[i][None, :], out=blk)

    # xpool: sum raw per-core partials into global graphs, then affine with
    # total graph sizes
    xpool = np.zeros((n_graphs, L * D), np.float32)
    for c in range(NCORES):
        span = int(meta["g_span"][c])
        lo = int(meta["g_lo"][c])
        part = results[c]["xpool_raw"]  # [L*128, 128]
        for i in range(L):
            xpool[lo:lo + span, i * D:(i + 1) * D] += part[i * P:i * P + span, :]
    gsizes = np.bincount(batch, minlength=n_graphs).astype(np.float32)
    for i in range(L):
        blk = xpool[:, i * D:(i + 1) * D]
        np.multiply(blk, a[i][None, :], out=blk)
        blk += gsizes[:, None] * t[i][None, :]

    commit = np.float32(sum(float(r["commit_out"].sum()) for r in results)
                        * COMMIT_W / (N * D))

    ids = []
    for r in results:
        ip = r["ids_pack"].reshape(P, NTILE, L * R)
        ids.append(ip.transpose(1, 0, 2).reshape(NTILE * P, L * R)[:NB])
    id_cat = np.concatenate(ids, axis=0).astype(np.int32)

    return xpool, xs_cat, np.float32(commit), id_cat
